# revision 7
# baseline (speedup 1.0000x reference)
"""GAT layer kernel for Trainium2, 8 NeuronCores (SPMD via run_bass_kernel_spmd).

Reference computation (N=8192, D_IN=512, D_OUT=256):
    h = input @ W; f1 = h @ a1; f2 = h @ a2
    e = leaky_relu(f1 + f2.T, 0.01); scores = where(adj>0, e, -9e15)
    att = softmax(scores, axis=1); out = elu(att @ h)

Strategy: row-shard the N nodes across 8 cores (1024 rows each). Each core:
  - replicates h = input@W (fp16 matmuls, [h | wa2-col] via augmented W)
  - computes its rows' attention weights TRANSPOSED (j on partitions, i free)
    as q = exp(leaky_relu(x + mask)) with an ADDITIVE mask: the host sends
    mm in {0, -40} in the 0.01-scaled logit domain, so for non-edges
    leaky(x-4000) = 0.01x - 40 and exp gives exactly 0.  Two alternating
    per-pr paths balance Vector vs Scalar load, both mathematically exact
    and with no ACT->DVE backedge (each path ends on the engine that
    produces q, so in-order queues never head-block on the other engine):
      path V1 (Scalar-heavy): u = 0.01f1+mm        [DVE tt x2]
                              t = prelu(100u + f2) [ACT x2, alpha=0.01]
                              q = exp(t)           [ACT, 2048 wide]
      path V3 (Vector-heavy): u = 0.01x+mm         [DVE ts x2 + tt]
                              t = max(u, 0.01u)    [DVE ts + tt]
                              q = exp(100t)        [ACT, 2048 wide]
  - accumulates out.T-free matmul: psum[i,:] += q_slice.T @ [h | ones]
    (ones column yields the softmax denominator for free)
  - normalizes rows + ELU, writes its [1024, 256] slice.
Softmax needs no max-subtraction: logits are bounded (~|x|<40) in fp32.
"""
import sys
import numpy as np

sys.path.insert(0, "/root/.axon_site/_ro/trn_rl_repo")
import ml_dtypes
from contextlib import ExitStack

from concourse import bass, tile, mybir, bacc
from concourse.bass_utils import run_bass_kernel_spmd

F32 = mybir.dt.float32
F16 = mybir.dt.float16
BF16 = mybir.dt.bfloat16
AF = mybir.ActivationFunctionType
ALU = mybir.AluOpType
BF = ml_dtypes.bfloat16

N, D_IN, D_OUT = 8192, 512, 256
NCORES = 8
ROWS = N // NCORES          # 1024 rows per core
JT = N // 128               # 64 j-tiles
DT = D_IN // 128            # 4 d-tiles
IT = ROWS // 128            # 8 i-tiles per core
HCOLS = 258                 # HB slot: 256 h + 2 ones (4B-aligned slots)
WCOLS = 258                 # W_aug: 256 W cols + wa2 + zero pad

_cache = {}


def path_v1(pr):
    """Scalar-heavy path for ~60% of prs (balances engine busy time)."""
    return pr % 5 in (0, 2, 4)


def _build():
    nc = bacc.Bacc("TRN2", target_bir_lowering=False, debug=False)

    d_inT = nc.dram_tensor("inT", [DT, 128, N], F16, kind="ExternalInput").ap()
    d_inOwn = nc.dram_tensor("inOwn", [DT, 128, ROWS], F16, kind="ExternalInput").ap()
    d_waug = nc.dram_tensor("waug", [DT, 128, WCOLS], F16, kind="ExternalInput").ap()
    d_wa1 = nc.dram_tensor("wa1", [DT, 128, 1], F16, kind="ExternalInput").ap()
    d_m = nc.dram_tensor("maskT", [JT // 2, 128, 2 * ROWS], F16, kind="ExternalInput").ap()
    d_out = nc.dram_tensor("out", [ROWS, D_OUT], F32, kind="ExternalOutput").ap()

    W2 = 2 * ROWS

    with tile.TileContext(nc) as tc, ExitStack() as ctx:
        const = ctx.enter_context(tc.tile_pool(name="const", bufs=1))
        # outer pool: attention elementwise tiles live across phase B and C
        p2 = ctx.enter_context(tc.tile_pool(name="p2", bufs=3))

        # ---- persistent SBUF tensors ----
        HB = const.tile([128, JT * HCOLS], BF16)          # [h | 1 | 1] per j-tile
        F2s = const.tile([128, JT], F32)                  # 0.01*f2 (V3 scalar)
        F2r = const.tile([128, JT], F32)                  # raw f2 (V1 bias)
        Waug = [const.tile([128, WCOLS], F16, name=f"waug{d}", tag=f"waug{d}") for d in range(DT)]
        wa1b = [const.tile([128, 128], F16, name=f"wa1b{d}", tag=f"wa1b{d}") for d in range(DT)]
        inOwn = [const.tile([128, ROWS], F16, name=f"inown{d}", tag=f"inown{d}") for d in range(DT)]
        f1b2 = const.tile([128, ROWS], F16)               # 0.01*f1 bcast
        thr = const.tile([128, 1], F16)                   # dma-throttle dummy

        qs = []          # q tiles produced in phase B, consumed by phase C

        # ---- phase 0: load weights (W_aug and wa1 prepped host-side) ----
        with tc.tile_pool(name="p0", bufs=2) as p0:
            for d in range(DT):
                nc.sync.dma_start(Waug[d][:], d_waug[d])
            for d in range(DT):
                t = p0.tile([128, 1], F16, tag="wa1c", name=f"wa1c{d}")
                nc.sync.dma_start(t[:], d_wa1[d])
                nc.vector.tensor_copy(wa1b[d][:], t[:].broadcast_to([128, 128]))
            for d in range(DT):
                nc.sync.dma_start(inOwn[d][:], d_inOwn[d])

        # ---- phase 1: f1 broadcast, then h = input @ [W | wa2] ----
        with tc.tile_pool(name="p1", bufs=6) as p1, \
             tc.tile_pool(name="psf", bufs=1, space="PSUM") as psf_pool, \
             tc.tile_pool(name="ps1", bufs=1, space="PSUM") as ps1:
            # f1 FIRST: everything in phase-2 elementwise depends on it
            psf = [psf_pool.tile([128, 512], F32, name=f"psf{h}", tag=f"psf{h}") for h in range(2)]
            for d in range(DT):
                for h in range(2):
                    nc.tensor.matmul(psf[h][:], wa1b[d][:],
                                     inOwn[d][:, 512 * h: 512 * (h + 1)],
                                     start=(d == 0), stop=(d == DT - 1))
            for h in range(2):
                sl = slice(512 * h, 512 * (h + 1))
                nc.vector.tensor_scalar(f1b2[:, sl], psf[h][:], 0.01, None,
                                        op0=ALU.mult)

            def emit_pr(pr):
                jt0 = 2 * pr
                m_t = p2.tile([128, W2], F16, tag="mask", bufs=5)
                nc.gpsimd.dma_start(m_t[:], d_m[pr])
                if path_v1(pr):
                    u_t = p2.tile([128, W2], F16, tag="uB", bufs=2)
                    for h in range(2):
                        sl = slice(h * ROWS, (h + 1) * ROWS)
                        nc.vector.tensor_tensor(u_t[:, sl], f1b2[:], m_t[:, sl],
                                                op=ALU.add)
                    t_t = p2.tile([128, W2], F16, tag="tB", bufs=2)
                    for h in range(2):
                        sl = slice(h * ROWS, (h + 1) * ROWS)
                        nc.scalar.activation(t_t[:, sl], u_t[:, sl], AF.Prelu,
                                             scale=100.0,
                                             bias=F2r[:, jt0 + h: jt0 + h + 1],
                                             alpha=0.01)
                    q_t = p2.tile([128, W2], BF16, tag="q", bufs=10)
                    nc.scalar.activation(q_t[:], t_t[:], AF.Exp)
                else:
                    # u = 0.01x + mm ; t = leaky(u) on DVE ; q = exp(100t)
                    u1 = p2.tile([128, W2], F16, tag="uA1", bufs=2)
                    for h in range(2):
                        sl = slice(h * ROWS, (h + 1) * ROWS)
                        nc.vector.tensor_scalar(u1[:, sl], f1b2[:],
                                                F2s[:, jt0 + h: jt0 + h + 1],
                                                None, op0=ALU.add)
                    u_t = p2.tile([128, W2], F16, tag="uA2", bufs=2)
                    nc.vector.tensor_tensor(u_t[:], u1[:], m_t[:], op=ALU.add)
                    us = p2.tile([128, W2], F16, tag="uA3", bufs=2)
                    nc.vector.tensor_scalar(us[:], u_t[:], 0.01, None,
                                            op0=ALU.mult)
                    t_t = p2.tile([128, W2], F16, tag="tA", bufs=2)
                    nc.vector.tensor_tensor(t_t[:], u_t[:], us[:], op=ALU.max)
                    q_t = p2.tile([128, W2], BF16, tag="q", bufs=10)
                    nc.scalar.activation(q_t[:], t_t[:], AF.Exp, scale=100.0)
                qs.append(q_t)

            for g in range(JT // 8):          # groups of 8 j-tiles
                it_g = []
                for d in range(DT):
                    t = p1.tile([128, 1024], F16, tag=f"instream{d}", bufs=3,
                                name=f"ing{d}_{g}")
                    nc.sync.dma_start(t[:], d_inT[d, :, 1024 * g: 1024 * (g + 1)])
                    it_g.append(t)
                # throttle: mask DMAs for this group's prs issue only after
                # this group's input stream has landed (keeps the critical
                # inT/weight loads ahead of bulk mask traffic in the DMA).
                nc.gpsimd.tensor_copy(thr[:], it_g[0][:, 0:1])
                for j8 in range(8):
                    jt = 8 * g + j8
                    psh = ps1.tile([128, WCOLS], F32, tag="psh", bufs=4)
                    for d in range(DT):
                        nc.tensor.matmul(psh[:], it_g[d][:, 128 * j8: 128 * (j8 + 1)],
                                         Waug[d][:],
                                         start=(d == 0), stop=(d == DT - 1))
                    nc.gpsimd.memset(HB[:, jt * HCOLS + D_OUT: jt * HCOLS + D_OUT + 2], 1.0)
                    nc.vector.tensor_copy(HB[:, jt * HCOLS: jt * HCOLS + D_OUT],
                                          psh[:, 0:D_OUT])
                    if path_v1(jt // 2):
                        nc.vector.tensor_copy(F2r[:, jt: jt + 1],
                                              psh[:, D_OUT:D_OUT + 1])
                    else:
                        nc.vector.tensor_scalar(F2s[:, jt: jt + 1],
                                                psh[:, D_OUT:D_OUT + 1],
                                                0.01, None, op0=ALU.mult)
                    if jt % 2 == 1:
                        emit_pr((jt - 1) // 2)

        # ---- phase 2: aggregation matmuls + tail ----
        with tc.tile_pool(name="psacc", bufs=1, space="PSUM") as psacc_pool, \
             tc.tile_pool(name="tail", bufs=2) as tail:
            acc = [psacc_pool.tile([128, WCOLS], F32, name=f"acc{k}", tag=f"acc{k}") for k in range(IT)]
            for pr in range(JT // 2):
                q_t = qs[pr]
                for h in range(2):
                    jt = 2 * pr + h
                    hb_j = HB[:, jt * HCOLS: jt * HCOLS + D_OUT + 2]
                    for k in range(IT):
                        nc.tensor.matmul(acc[k][:],
                                         q_t[:, h * ROWS + 128 * k: h * ROWS + 128 * (k + 1)],
                                         hb_j,
                                         start=(jt == 0), stop=(jt == JT - 1))

            # ---- tail: normalize + ELU + store ----
            for k in range(IT):
                r = tail.tile([128, 1], F32, tag="r")
                nc.vector.reciprocal(r[:], acc[k][:, D_OUT:D_OUT + 1])
                x = tail.tile([128, D_OUT], F32, tag="x")
                nc.scalar.activation(x[:], acc[k][:, 0:D_OUT], AF.Copy,
                                     scale=r[:])
                u = tail.tile([128, D_OUT], F32, tag="u2")
                nc.vector.tensor_scalar(u[:], x[:], 0.0, None, op0=ALU.min)
                v = tail.tile([128, D_OUT], F32, tag="v")
                nc.scalar.activation(v[:], u[:], AF.Exp)
                o = tail.tile([128, D_OUT], F32, tag="o")
                nc.vector.scalar_tensor_tensor(o[:], v[:], -1.0, x[:],
                                               op0=ALU.add, op1=ALU.max)
                nc.sync.dma_start(d_out[128 * k: 128 * (k + 1), :], o[:])

    nc.compile()
    return nc


def _prep_inputs(input, adj, W, a1, a2):
    inputT = np.ascontiguousarray(input.T).astype(np.float16)   # [512, 8192]
    inT = inputT.reshape(DT, 128, N)
    W16 = W.astype(np.float16)
    wa = (W16.astype(np.float32) @ np.concatenate([a1, a2], axis=1).astype(np.float32))
    waug = np.zeros((D_IN, WCOLS), np.float16)
    waug[:, 0:D_OUT] = W16
    waug[:, D_OUT] = wa[:, 1].astype(np.float16)
    waug = waug.reshape(DT, 128, WCOLS)
    wa1c = wa[:, 0].astype(np.float16).reshape(DT, 128, 1)
    shared = {"inT": inT, "waug": waug, "wa1": wa1c}

    in_maps = []
    for c in range(NCORES):
        r0 = c * ROWS
        maskT = np.where(adj[r0:r0 + ROWS, :] != 0,
                         np.float16(0.0), np.float16(-40.0)).T   # [8192, 1024]
        maskT = (np.ascontiguousarray(maskT).reshape(JT // 2, 2, 128, ROWS)
                 .transpose(0, 2, 1, 3).reshape(JT // 2, 128, 2 * ROWS).copy())
        own = np.ascontiguousarray(inputT[:, r0:r0 + ROWS]).reshape(DT, 128, ROWS)
        in_maps.append({**shared, "inOwn": own, "maskT": maskT})
    return in_maps


def run(inputs: dict, trace: bool = False):
    if "nc" not in _cache:
        _cache["nc"] = _build()
    nc = _cache["nc"]
    in_maps = _prep_inputs(inputs["input"], inputs["adj"],
                           inputs["W"], inputs["a1"], inputs["a2"])
    res = run_bass_kernel_spmd(nc, in_maps, core_ids=list(range(NCORES)),
                               trace=trace)
    out = np.concatenate([res.results[c]["out"] for c in range(NCORES)], axis=0)
    return out, res


def kernel(**inputs) -> np.ndarray:
    out, _ = run(inputs)
    return out
